# revision 35
# baseline (speedup 1.0000x reference)
"""Trainium2 Bass kernel: basic GCN layer, row-parallel over 8 NeuronCores.

    Y = relu( D^-1/2 (A + I) D^-1/2 (H @ W.T + b) ),  D = (A + I).sum(axis=1)

Sharding: core i owns output rows [i*1024, (i+1)*1024).

v5 design:
- A[rows].T is stored fp8 (binary, lossless) in a pre-tiled DRAM layout so
  every load is a [128, contiguous] block copy at full HBM rate, and kept
  fp8-resident in SBUF (64 KiB/partition).
- Row sums run on the PE in fp8 DoubleRow mode (2 k-tiles/instruction, 2x
  element rate) chasing the A DMA; per-r-half raw sums are all-gathered
  (ncores x 512 f32) as soon as that half's rows are summed. A dummy
  0-byte-ish AllGather is fired at kernel start so the ncfw/mesh first-call
  overhead is paid during the load window instead of on the critical path.
- X = D^-1/2 (H W^T + b) in bf16; the bias is added by the DVE during the
  PSUM->SBUF copy (broadcast tile), not by PE matmuls.
- The main product Y[rows].T uses mixed-dtype matmuls (bf16 X stationary,
  fp8 A moving), k-tiles emitted in the order their all-gathered D^-1/2
  chunk lands.
"""

import os
import sys

import numpy as np

for _p in ("/opt/trn_rl_repo", "/root/.axon_site/_ro/trn_rl_repo"):
    if _p not in sys.path and os.path.isdir(_p):
        sys.path.insert(0, _p)

N = 8192        # nodes
NCORES = 8
RPC = N // NCORES  # rows per core (1024)
P = 128         # partitions / tile edge
F = 128         # feature dim (in == out)


def _build_nc(n=8192, rpc=1024, f=128, ncores=8, warm0=10, warm1=10):
    import concourse.bass as bass  # noqa: F401
    import concourse.mybir as mybir
    from concourse import bacc, tile
    from concourse.masks import make_identity

    dt = mybir.dt
    f32, bf, f8 = dt.float32, dt.bfloat16, dt.float8e4

    P = 128
    kt = n // P                 # c-tiles (contraction dim), 64
    kpc = kt // ncores          # c-tiles per core's row range, 8
    NRC = 2                     # r-halves
    RC = rpc // NRC             # rows per half, 512
    khalf = kpc // NRC          # c-tiles per (core, half), 4
    GR = ncores * RC // P       # gathered [GR, 128] rows per half, 32
    HB = kt * RC                # A bytes/partition per half (32768)
    NG = 4                      # A DMA groups per half (1 MB each)
    GB = HB // NG               # bytes/partition per group (8192)

    def k_set(h):
        return [k for k in range(kt) if (k % kpc) // khalf == h]

    def j_idx(k, h):
        return khalf * (k // kpc) + (k % kpc) - h * khalf

    nc = bacc.Bacc("TRN2", num_devices=ncores)

    # a2[p, h*HB + k*RC + r] = A[row i*rpc + h*RC + r, col k*128 + p]
    a2 = nc.dram_tensor("a2", [P, NRC * HB], f8, kind="ExternalInput")
    ht = nc.dram_tensor("ht", [f, n], bf, kind="ExternalInput")          # H.T
    hlt = nc.dram_tensor("hlt", [f, rpc], bf, kind="ExternalInput")      # H[rows].T
    wt = nc.dram_tensor("wt", [f, f], bf, kind="ExternalInput")          # W.T
    bias = nc.dram_tensor("bias", [1, 4 * f], bf, kind="ExternalInput")  # b x4
    out = nc.dram_tensor("out", [f, rpc], f32, kind="ExternalOutput")    # Y[rows].T

    with tile.TileContext(nc) as tc:
        with (
            tc.tile_pool(name="const", bufs=1) as cpool,
            tc.tile_pool(name="abuf", bufs=1) as apool,
            tc.tile_pool(name="xbuf", bufs=1) as xpool,
            tc.tile_pool(name="work", bufs=1) as wpool,
            tc.tile_pool(name="tmp", bufs=2) as tpool,
            tc.tile_pool(name="pshw", bufs=2, space="PSUM") as pshw,
            tc.tile_pool(name="psbig", bufs=1, space="PSUM") as psbig,
            tc.tile_pool(name="dram", bufs=1, space="DRAM") as dpool,
        ):
            wt_sb = cpool.tile([f, f], bf, tag="wt", name="wt_sb")
            bias_sb = cpool.tile([1, 4 * f], bf, tag="bias", name="bias_sb")
            bias_bc = cpool.tile([P, 4 * f], bf, tag="biasbc", name="bias_bc")
            hlt_sb = cpool.tile([f, rpc], bf, tag="hlt", name="hlt_sb")
            ht_sb = cpool.tile([f, n], bf, tag="ht", name="ht_sb")
            # DoubleRow row-sum weights: [K=128, 2 pair, M=128] selector with
            # column m=0 all-ones -> output row 0 = column sums of the pair
            esel = cpool.tile([P, 2 * P], f8, tag="esel", name="esel")
            ones_r = cpool.tile([1, 512], bf, tag="onesr", name="ones_r")
            ident = cpool.tile([P, P], f32, tag="ident", name="ident")
            # A striped across BOTH HWDGE rings as one 2 MB chunk per
            # (ring, half) -- per-dma ring-processing gaps were costing
            # ~2-3 us each. h0 chunks first so the first all-gather fires
            # early; consts AFTER the A chunks on the scalar ring (their
            # consumers idle until ~20 us anyway); H.T last on sync.
            a_sb = apool.tile([P, NRC * HB], f8, tag="a", name="a_sb")
            HH = HB // 2
            for h in range(NRC):
                off = h * HB
                nc.sync.dma_start(a_sb[:, off:off + HH], a2[:, off:off + HH])
                nc.scalar.dma_start(a_sb[:, off + HH:off + HB],
                                    a2[:, off + HH:off + HB])
            nc.sync.dma_start(ht_sb[:], ht[:])
            nc.scalar.dma_start(wt_sb[:], wt[:])
            nc.scalar.dma_start(bias_sb[:], bias[:])
            nc.scalar.dma_start(hlt_sb[:], hlt[:])
            nc.scalar.dma_start(
                bias_bc[:].rearrange("p (o m) -> p o m", o=1),
                bias[0:1, :].partition_broadcast(P),
            )
            nc.vector.memset(esel[:], 0.0)
            nc.vector.memset(esel[:, 0:1], 1.0)
            nc.vector.memset(esel[:, P:P + 1], 1.0)
            nc.vector.memset(ones_r[:], 1.0)
            make_identity(nc, ident[:])

            def a_slice(k, h):
                off = h * HB + k * RC
                return a_sb[:, off:off + RC]

            def a_pair(m, h):
                off = h * HB + 2 * m * RC
                return a_sb[:, off:off + 2 * RC].rearrange(
                    "p (two r) -> p two r", two=2)

            esel_ap = esel[:].rearrange("p (two m) -> p two m", two=2)

            # PE clock warm-up at kernel start
            ps_warm = pshw.tile([1, 512], f32, tag="hw", name="ps_warm0")
            for _ in range(warm0):
                nc.tensor.matmul(ps_warm[0:1, :], ones_r[0:1, 0:1],
                                 ones_r[0:1, :], start=True, stop=True)

            # ---- HWl.T (+bias) for the self-loop term ----
            ps_hwl = [psbig.tile([f, RC], f32, tag=f"hwl{h}", name=f"hwl{h}")
                      for h in range(NRC)]
            for h in range(NRC):
                nc.tensor.matmul(ps_hwl[h][:, :], wt_sb[:, :],
                                 hlt_sb[:, h * RC:(h + 1) * RC],
                                 start=True, stop=False)
                nc.tensor.matmul(ps_hwl[h][:, :], bias_sb[0:1, 0:f],
                                 ones_r[0:1, 0:RC], start=False, stop=True)

            # ---- per-half: fp8 DoubleRow row sums -> all-gather raw sums ----
            ps_rs = [psbig.tile([P, RC], f32, tag=f"rs{h}", name=f"rs{h}")
                     for h in range(NRC)]
            rs_sb, cc_in, cc_out = [], [], []
            for h in range(NRC):
                for m in range(kt // 2):
                    nc.tensor.matmul(
                        ps_rs[h][:, :], esel_ap, a_pair(m, h),
                        start=(m == 0), stop=(m == kt // 2 - 1),
                        perf_mode=mybir.MatmulPerfMode.DoubleRow,
                    )
                rs = wpool.tile([1, RC], f32, tag=f"rs_sb{h}", name=f"rs_sb{h}")
                # D = rowsum(A) + 1 (self loop) folded into the PSUM copy
                nc.vector.tensor_scalar_add(rs[0:1, :], ps_rs[h][0:1, :], 1.0)
                ci = dpool.tile([1, RC], f32, tag=f"ccin{h}", name=f"cc_in{h}")
                co = dpool.tile([ncores, RC], f32, tag=f"ccout{h}",
                                name=f"cc_out{h}", addr_space="Shared")
                nc.scalar.dma_start(ci[:], rs[:])
                nc.gpsimd.collective_compute(
                    "AllGather", mybir.AluOpType.bypass,
                    replica_groups=[list(range(ncores))],
                    ins=[ci.opt()], outs=[co.opt()],
                )
                rs_sb.append(rs)
                cc_in.append(ci)
                cc_out.append(co)

            # ---- local D^-1/2 broadcast: dlb[p, r] = 1/sqrt(D[r]) ----
            dlb = wpool.tile([P, rpc], f32, tag="dlb", name="dlb")
            for h in range(NRC):
                sl = dlb[:, h * RC:(h + 1) * RC]
                nc.scalar.dma_start(
                    sl.rearrange("p (o r) -> p o r", o=1),
                    cc_in[h][0:1, :].partition_broadcast(P),
                )
                nc.scalar.sqrt(sl, sl)
                nc.vector.reciprocal(sl, sl)

            # ---- unscaled HW+b = H @ W.T + b (PE fills the load window) ----
            # 4 k-tiles batched per PSUM bank so one wide DVE add drains 4
            # matmuls (per-instruction DVE overhead was serializing this)
            hw4 = []
            for q in range(kt // 4):
                ps_hw = pshw.tile([P, 4 * f], f32, tag="hw", name=f"hw{q}")
                for s in range(4):
                    k = 4 * q + s
                    nc.tensor.matmul(ps_hw[:, s * f:(s + 1) * f],
                                     ht_sb[:, k * P:(k + 1) * P],
                                     wt_sb[:, :], start=True, stop=True)
                hq = xpool.tile([P, 4 * f], bf, tag=f"hw_nb{q}",
                                name=f"hw_nb{q}")
                # bias add fused into the PSUM -> SBUF copy
                nc.vector.tensor_add(hq[:, :], ps_hw[:, :], bias_bc[:, :])
                hw4.append(hq)

            def hw_slice(k):
                return hw4[k // 4][:, (k % 4) * f:(k % 4 + 1) * f]

            # ---- PE warm-keeper before the AG stall ----
            for _ in range(warm1):
                nc.tensor.matmul(ps_warm[0:1, :], ones_r[0:1, 0:1],
                                 ones_r[0:1, :], start=True, stop=True)

            # ---- gathered sums -> dinv[h][p, j] = 1/sqrt(s[128j + p]) ----
            dinv = []
            for h in range(NRC):
                rs2d = wpool.tile([GR, P], f32, tag=f"rs2d{h}", name=f"rs2d{h}")
                nc.scalar.dma_start(
                    rs2d[:], cc_out[h][:].rearrange("g (m p) -> (g m) p", p=P))
                ps_t = pshw.tile([P, GR], f32, tag="hw", name=f"ps_t{h}")
                nc.tensor.transpose(ps_t[:, :], rs2d[:, :], ident[0:GR, 0:GR])
                dv = wpool.tile([P, GR], f32, tag=f"dinv{h}", name=f"dinv{h}")
                nc.scalar.sqrt(dv[:, :], ps_t[:, :])
                nc.vector.reciprocal(dv[:, :], dv[:, :])
                dinv.append(dv)

            # ---- main matmul: bf16 X stationary x fp8 A moving ----
            ps_main = [psbig.tile([f, RC], f32, tag=f"main{h}", name=f"main{h}")
                       for h in range(NRC)]
            emit_order = [(hs, k) for hs in range(NRC) for k in k_set(hs)]
            for idx, (hs, k) in enumerate(emit_order):
                j = j_idx(k, hs)
                nc.vector.tensor_scalar_mul(hw_slice(k), hw_slice(k),
                                            dinv[hs][:, j:j + 1])
                for h in range(NRC):
                    nc.tensor.matmul(
                        ps_main[h][:, :], hw_slice(k), a_slice(k, h),
                        start=(idx == 0), stop=(idx == kt - 1),
                    )

            # ---- epilogue: Y.T = relu(dl * (main + dl * HWl.T)) ----
            y_sb = wpool.tile([f, rpc], f32, tag="y", name="y_sb")
            for h in range(NRC):
                dlb_hh = dlb[:, h * RC:(h + 1) * RC]
                t1 = tpool.tile([f, RC], f32, tag="t1", name=f"t1_{h}")
                nc.vector.tensor_mul(t1[:, :], ps_hwl[h][:, :], dlb_hh)
                t2 = tpool.tile([f, RC], f32, tag="t2", name=f"t2_{h}")
                nc.vector.tensor_add(t2[:, :], ps_main[h][:, :], t1[:, :])
                t3 = tpool.tile([f, RC], f32, tag="t3", name=f"t3_{h}")
                nc.vector.tensor_mul(t3[:, :], t2[:, :], dlb_hh)
                nc.vector.tensor_scalar_max(y_sb[:, h * RC:(h + 1) * RC],
                                            t3[:, :], 0.0)
                nc.scalar.dma_start(out[:, h * RC:(h + 1) * RC],
                                    y_sb[:, h * RC:(h + 1) * RC])

    nc.compile()
    return nc


_CACHE = {}


def _get_nc():
    if "nc" not in _CACHE:
        _CACHE["nc"] = _build_nc()
    return _CACHE["nc"]


def _prep_in_maps(H, A, W, b):
    import ml_dtypes

    bf16 = ml_dtypes.bfloat16
    fp8 = ml_dtypes.float8_e4m3
    H = np.asarray(H, dtype=np.float32)
    A = np.asarray(A, dtype=np.float32)
    W = np.asarray(W, dtype=np.float32)
    b = np.asarray(b, dtype=np.float32)
    Hb = H.astype(bf16)
    ht = np.ascontiguousarray(Hb.T)
    wt = np.ascontiguousarray(W.T.astype(bf16))
    bias = np.ascontiguousarray(np.tile(b, 4).reshape(1, -1).astype(bf16))
    maps = []
    for i in range(NCORES):
        rows = slice(i * RPC, (i + 1) * RPC)
        # [p, h, k, r] <- A[row h*512+r, col k*128+p]
        Ac = A[rows, :].reshape(2, 512, 64, 128).transpose(3, 0, 2, 1)
        a2 = np.ascontiguousarray(Ac.reshape(128, 65536).astype(fp8))
        maps.append({
            "a2": a2,
            "ht": ht,
            "hlt": np.ascontiguousarray(Hb[rows, :].T),
            "wt": wt,
            "bias": bias,
        })
    return maps


def run(H, A, W, b, trace=False):
    from concourse import bass_utils

    nc = _get_nc()
    res = bass_utils.run_bass_kernel_spmd(
        nc, _prep_in_maps(H, A, W, b), core_ids=list(range(NCORES)),
        trace=trace,
    )
    Y = np.concatenate(
        [np.asarray(res.results[i]["out"]).T for i in range(NCORES)], axis=0
    )
    return np.ascontiguousarray(Y, dtype=np.float32), res


def kernel(H, A, W, b):
    return run(H, A, W, b)[0]


# revision 36
# speedup vs baseline: 1.1812x; 1.1812x over previous
"""Trainium2 Bass kernel: basic GCN layer, row-parallel over 8 NeuronCores.

    Y = relu( D^-1/2 (A + I) D^-1/2 (H @ W.T + b) ),  D = (A + I).sum(axis=1)

Sharding: core i owns output rows [i*1024, (i+1)*1024).

v5 design:
- A[rows].T is stored fp8 (binary, lossless) in a pre-tiled DRAM layout so
  every load is a [128, contiguous] block copy at full HBM rate, and kept
  fp8-resident in SBUF (64 KiB/partition).
- Row sums run on the PE in fp8 DoubleRow mode (2 k-tiles/instruction, 2x
  element rate) chasing the A DMA; per-r-half raw sums are all-gathered
  (ncores x 512 f32) as soon as that half's rows are summed. A dummy
  0-byte-ish AllGather is fired at kernel start so the ncfw/mesh first-call
  overhead is paid during the load window instead of on the critical path.
- X = D^-1/2 (H W^T + b) in bf16; the bias is added by the DVE during the
  PSUM->SBUF copy (broadcast tile), not by PE matmuls.
- The main product Y[rows].T uses mixed-dtype matmuls (bf16 X stationary,
  fp8 A moving), k-tiles emitted in the order their all-gathered D^-1/2
  chunk lands.
"""

import os
import sys

import numpy as np

for _p in ("/opt/trn_rl_repo", "/root/.axon_site/_ro/trn_rl_repo"):
    if _p not in sys.path and os.path.isdir(_p):
        sys.path.insert(0, _p)

N = 8192        # nodes
NCORES = 8
RPC = N // NCORES  # rows per core (1024)
P = 128         # partitions / tile edge
F = 128         # feature dim (in == out)


def _build_nc(n=8192, rpc=1024, f=128, ncores=8, warm0=10, warm1=10):
    import concourse.bass as bass  # noqa: F401
    import concourse.mybir as mybir
    from concourse import bacc, tile
    from concourse.masks import make_identity

    dt = mybir.dt
    f32, bf, f8 = dt.float32, dt.bfloat16, dt.float8e4

    P = 128
    kt = n // P                 # c-tiles (contraction dim), 64
    kpc = kt // ncores          # c-tiles per core's row range, 8
    NRC = 2                     # r-halves
    RC = rpc // NRC             # rows per half, 512
    khalf = kpc // NRC          # c-tiles per (core, half), 4
    GR = ncores * RC // P       # gathered [GR, 128] rows per half, 32
    HB = kt * RC                # A bytes/partition per half (32768)
    NG = 4                      # A DMA groups per half (1 MB each)
    GB = HB // NG               # bytes/partition per group (8192)

    def k_set(h):
        return [k for k in range(kt) if (k % kpc) // khalf == h]

    def j_idx(k, h):
        return khalf * (k // kpc) + (k % kpc) - h * khalf

    nc = bacc.Bacc("TRN2", num_devices=ncores)

    # a2[p, h*HB + k*RC + r] = A[row i*rpc + h*RC + r, col k*128 + p]
    a2 = nc.dram_tensor("a2", [P, NRC * HB], f8, kind="ExternalInput")
    ht = nc.dram_tensor("ht", [f, n], bf, kind="ExternalInput")          # H.T
    hlt = nc.dram_tensor("hlt", [f, rpc], bf, kind="ExternalInput")      # H[rows].T
    wt = nc.dram_tensor("wt", [f, f], bf, kind="ExternalInput")          # W.T
    bias = nc.dram_tensor("bias", [1, 4 * f], bf, kind="ExternalInput")  # b x4
    out = nc.dram_tensor("out", [f, rpc], f32, kind="ExternalOutput")    # Y[rows].T

    with tile.TileContext(nc) as tc:
        with (
            tc.tile_pool(name="const", bufs=1) as cpool,
            tc.tile_pool(name="abuf", bufs=1) as apool,
            tc.tile_pool(name="xbuf", bufs=1) as xpool,
            tc.tile_pool(name="work", bufs=1) as wpool,
            tc.tile_pool(name="tmp", bufs=2) as tpool,
            tc.tile_pool(name="pshw", bufs=2, space="PSUM") as pshw,
            tc.tile_pool(name="psbig", bufs=1, space="PSUM") as psbig,
            tc.tile_pool(name="dram", bufs=1, space="DRAM") as dpool,
        ):
            wt_sb = cpool.tile([f, f], bf, tag="wt", name="wt_sb")
            bias_sb = cpool.tile([1, 4 * f], bf, tag="bias", name="bias_sb")
            bias_bc = cpool.tile([P, 4 * f], bf, tag="biasbc", name="bias_bc")
            hlt_sb = cpool.tile([f, rpc], bf, tag="hlt", name="hlt_sb")
            ht_sb = cpool.tile([f, n], bf, tag="ht", name="ht_sb")
            # DoubleRow row-sum weights: [K=128, 2 pair, M=128] selector with
            # column m=0 all-ones -> output row 0 = column sums of the pair
            esel = cpool.tile([P, 2 * P], f8, tag="esel", name="esel")
            ones_r = cpool.tile([1, 512], bf, tag="onesr", name="ones_r")
            ident = cpool.tile([P, P], f32, tag="ident", name="ident")
            # small consts on the scalar (ACT HWDGE) ring
            nc.scalar.dma_start(wt_sb[:], wt[:])
            nc.scalar.dma_start(bias_sb[:], bias[:])
            nc.scalar.dma_start(hlt_sb[:], hlt[:])
            nc.scalar.dma_start(
                bias_bc[:].rearrange("p (o m) -> p o m", o=1),
                bias[0:1, :].partition_broadcast(P),
            )
            # A striped across BOTH HWDGE rings (each ring adds per-DMA
            # processing gaps; two rings pipeline them away), h0 groups
            # first on both rings so the first all-gather can fire early.
            # H.T last on the sync ring: its consumers (X precompute) only
            # gate the main matmul, which waits for the all-gather anyway.
            a_sb = apool.tile([P, NRC * HB], f8, tag="a", name="a_sb")
            for h in range(NRC):
                for g in range(NG):
                    off = h * HB + g * GB
                    eng = nc.sync if g % 2 == 0 else nc.scalar
                    eng.dma_start(a_sb[:, off:off + GB],
                                  a2[:, off:off + GB])
            nc.sync.dma_start(ht_sb[:], ht[:])
            nc.vector.memset(esel[:], 0.0)
            nc.vector.memset(esel[:, 0:1], 1.0)
            nc.vector.memset(esel[:, P:P + 1], 1.0)
            nc.vector.memset(ones_r[:], 1.0)
            make_identity(nc, ident[:])

            def a_slice(k, h):
                off = h * HB + k * RC
                return a_sb[:, off:off + RC]

            def a_pair(m, h):
                off = h * HB + 2 * m * RC
                return a_sb[:, off:off + 2 * RC].rearrange(
                    "p (two r) -> p two r", two=2)

            esel_ap = esel[:].rearrange("p (two m) -> p two m", two=2)

            # PE clock warm-up at kernel start
            ps_warm = pshw.tile([1, 512], f32, tag="hw", name="ps_warm0")
            for _ in range(warm0):
                nc.tensor.matmul(ps_warm[0:1, :], ones_r[0:1, 0:1],
                                 ones_r[0:1, :], start=True, stop=True)

            # ---- HWl.T (+bias) for the self-loop term ----
            ps_hwl = [psbig.tile([f, RC], f32, tag=f"hwl{h}", name=f"hwl{h}")
                      for h in range(NRC)]
            for h in range(NRC):
                nc.tensor.matmul(ps_hwl[h][:, :], wt_sb[:, :],
                                 hlt_sb[:, h * RC:(h + 1) * RC],
                                 start=True, stop=False)
                nc.tensor.matmul(ps_hwl[h][:, :], bias_sb[0:1, 0:f],
                                 ones_r[0:1, 0:RC], start=False, stop=True)

            # ---- per-half: fp8 DoubleRow row sums -> all-gather raw sums ----
            ps_rs = [psbig.tile([P, RC], f32, tag=f"rs{h}", name=f"rs{h}")
                     for h in range(NRC)]
            rs_sb, cc_in, cc_out = [], [], []
            for h in range(NRC):
                for m in range(kt // 2):
                    nc.tensor.matmul(
                        ps_rs[h][:, :], esel_ap, a_pair(m, h),
                        start=(m == 0), stop=(m == kt // 2 - 1),
                        perf_mode=mybir.MatmulPerfMode.DoubleRow,
                    )
                rs = wpool.tile([1, RC], f32, tag=f"rs_sb{h}", name=f"rs_sb{h}")
                # D = rowsum(A) + 1 (self loop) folded into the PSUM copy
                nc.vector.tensor_scalar_add(rs[0:1, :], ps_rs[h][0:1, :], 1.0)
                ci = dpool.tile([1, RC], f32, tag=f"ccin{h}", name=f"cc_in{h}")
                co = dpool.tile([ncores, RC], f32, tag=f"ccout{h}",
                                name=f"cc_out{h}", addr_space="Shared")
                nc.scalar.dma_start(ci[:], rs[:])
                nc.gpsimd.collective_compute(
                    "AllGather", mybir.AluOpType.bypass,
                    replica_groups=[list(range(ncores))],
                    ins=[ci.opt()], outs=[co.opt()],
                )
                rs_sb.append(rs)
                cc_in.append(ci)
                cc_out.append(co)

            # ---- local D^-1/2 broadcast: dlb[p, r] = 1/sqrt(D[r]) ----
            dlb = wpool.tile([P, rpc], f32, tag="dlb", name="dlb")
            for h in range(NRC):
                sl = dlb[:, h * RC:(h + 1) * RC]
                nc.scalar.dma_start(
                    sl.rearrange("p (o r) -> p o r", o=1),
                    cc_in[h][0:1, :].partition_broadcast(P),
                )
                nc.scalar.sqrt(sl, sl)
                nc.vector.reciprocal(sl, sl)

            # ---- unscaled HW+b = H @ W.T + b (PE fills the load window) ----
            # 4 k-tiles batched per PSUM bank so one wide DVE add drains 4
            # matmuls (per-instruction DVE overhead was serializing this)
            hw4 = []
            for q in range(kt // 4):
                ps_hw = pshw.tile([P, 4 * f], f32, tag="hw", name=f"hw{q}")
                for s in range(4):
                    k = 4 * q + s
                    nc.tensor.matmul(ps_hw[:, s * f:(s + 1) * f],
                                     ht_sb[:, k * P:(k + 1) * P],
                                     wt_sb[:, :], start=True, stop=True)
                hq = xpool.tile([P, 4 * f], bf, tag=f"hw_nb{q}",
                                name=f"hw_nb{q}")
                # bias add fused into the PSUM -> SBUF copy
                nc.vector.tensor_add(hq[:, :], ps_hw[:, :], bias_bc[:, :])
                hw4.append(hq)

            def hw_slice(k):
                return hw4[k // 4][:, (k % 4) * f:(k % 4 + 1) * f]

            # ---- PE warm-keeper before the AG stall ----
            for _ in range(warm1):
                nc.tensor.matmul(ps_warm[0:1, :], ones_r[0:1, 0:1],
                                 ones_r[0:1, :], start=True, stop=True)

            # ---- gathered sums -> dinv[h][p, j] = 1/sqrt(s[128j + p]) ----
            dinv = []
            for h in range(NRC):
                rs2d = wpool.tile([GR, P], f32, tag=f"rs2d{h}", name=f"rs2d{h}")
                nc.scalar.dma_start(
                    rs2d[:], cc_out[h][:].rearrange("g (m p) -> (g m) p", p=P))
                ps_t = pshw.tile([P, GR], f32, tag="hw", name=f"ps_t{h}")
                nc.tensor.transpose(ps_t[:, :], rs2d[:, :], ident[0:GR, 0:GR])
                dv = wpool.tile([P, GR], f32, tag=f"dinv{h}", name=f"dinv{h}")
                nc.scalar.sqrt(dv[:, :], ps_t[:, :])
                nc.vector.reciprocal(dv[:, :], dv[:, :])
                dinv.append(dv)

            # ---- main matmul: bf16 X stationary x fp8 A moving ----
            ps_main = [psbig.tile([f, RC], f32, tag=f"main{h}", name=f"main{h}")
                       for h in range(NRC)]
            emit_order = [(hs, k) for hs in range(NRC) for k in k_set(hs)]
            for idx, (hs, k) in enumerate(emit_order):
                j = j_idx(k, hs)
                nc.vector.tensor_scalar_mul(hw_slice(k), hw_slice(k),
                                            dinv[hs][:, j:j + 1])
                for h in range(NRC):
                    nc.tensor.matmul(
                        ps_main[h][:, :], hw_slice(k), a_slice(k, h),
                        start=(idx == 0), stop=(idx == kt - 1),
                    )

            # ---- epilogue: Y.T = relu(dl * (main + dl * HWl.T)) ----
            y_sb = wpool.tile([f, rpc], f32, tag="y", name="y_sb")
            for h in range(NRC):
                dlb_hh = dlb[:, h * RC:(h + 1) * RC]
                t1 = tpool.tile([f, RC], f32, tag="t1", name=f"t1_{h}")
                nc.vector.tensor_mul(t1[:, :], ps_hwl[h][:, :], dlb_hh)
                t2 = tpool.tile([f, RC], f32, tag="t2", name=f"t2_{h}")
                nc.vector.tensor_add(t2[:, :], ps_main[h][:, :], t1[:, :])
                t3 = tpool.tile([f, RC], f32, tag="t3", name=f"t3_{h}")
                nc.vector.tensor_mul(t3[:, :], t2[:, :], dlb_hh)
                nc.vector.tensor_scalar_max(y_sb[:, h * RC:(h + 1) * RC],
                                            t3[:, :], 0.0)
                nc.scalar.dma_start(out[:, h * RC:(h + 1) * RC],
                                    y_sb[:, h * RC:(h + 1) * RC])

    nc.compile()
    return nc


_CACHE = {}


def _get_nc():
    if "nc" not in _CACHE:
        _CACHE["nc"] = _build_nc()
    return _CACHE["nc"]


def _prep_in_maps(H, A, W, b):
    import ml_dtypes

    bf16 = ml_dtypes.bfloat16
    fp8 = ml_dtypes.float8_e4m3
    H = np.asarray(H, dtype=np.float32)
    A = np.asarray(A, dtype=np.float32)
    W = np.asarray(W, dtype=np.float32)
    b = np.asarray(b, dtype=np.float32)
    Hb = H.astype(bf16)
    ht = np.ascontiguousarray(Hb.T)
    wt = np.ascontiguousarray(W.T.astype(bf16))
    bias = np.ascontiguousarray(np.tile(b, 4).reshape(1, -1).astype(bf16))
    maps = []
    for i in range(NCORES):
        rows = slice(i * RPC, (i + 1) * RPC)
        # [p, h, k, r] <- A[row h*512+r, col k*128+p]
        Ac = A[rows, :].reshape(2, 512, 64, 128).transpose(3, 0, 2, 1)
        a2 = np.ascontiguousarray(Ac.reshape(128, 65536).astype(fp8))
        maps.append({
            "a2": a2,
            "ht": ht,
            "hlt": np.ascontiguousarray(Hb[rows, :].T),
            "wt": wt,
            "bias": bias,
        })
    return maps


def run(H, A, W, b, trace=False):
    from concourse import bass_utils

    nc = _get_nc()
    res = bass_utils.run_bass_kernel_spmd(
        nc, _prep_in_maps(H, A, W, b), core_ids=list(range(NCORES)),
        trace=trace,
    )
    Y = np.concatenate(
        [np.asarray(res.results[i]["out"]).T for i in range(NCORES)], axis=0
    )
    return np.ascontiguousarray(Y, dtype=np.float32), res


def kernel(H, A, W, b):
    return run(H, A, W, b)[0]
